# revision 2
# baseline (speedup 1.0000x reference)
"""Distributed Bass kernel for nn_Attention_32701880992127 on 8 TRN2 NeuronCores.

Sharding (tensor parallel over heads): core d owns q-heads {2d, 2d+1} and
kv-head d//2 (GQA consecutive-repeat mapping). wq/wk/wv are column-split,
wo is split along its OUTPUT dim so each core produces a distinct column
slice of the final output from the AllGathered attention features (cheaper
than the all-reduce variant: 1MB/core gathered vs 16MB reduced).

All matmuls run in bf16 (f32 PSUM accumulation); elementwise math stays f32.
Softmax needs no max-subtraction (qk-norm bounds the scores), and the sink
correction folds into the denominator:
    out_h = (sum_k exp(s_qk) v_k) / (exp(sink_h) + sum_k exp(s_qk)).
Scores are computed transposed (ST[k, q]) so exp's output directly feeds the
PV matmul as the moving operand. The causal diagonal 512-block is processed
as 4 k-chunks with shrinking q-windows (1280 instead of 2048 columns per
group); only the 128x128 diagonal micro-block of each chunk needs a mask.
qT/kT are produced by XBAR DMA transposes (off the PE). Emission order keeps
the PE FIFO free of collective-dependent work until all local attention is
done: proj tiles + attention groups first, AllGathers fired as soon as each
q-group's features are shipped, all wo parts at the end.
"""
import numpy as np
import ml_dtypes

import concourse.mybir as mybir
import concourse.tile as tile
from concourse import bacc
from concourse.bass_utils import run_bass_kernel_spmd

dt = mybir.dt
AO = mybir.AluOpType
AF = mybir.ActivationFunctionType
BF16 = ml_dtypes.bfloat16

N_CORES = 8
S = 2048            # sequence length
D = 2048            # model dim
DH = 128            # head dim
HL = 2              # local q heads per core
NC = 16             # d-chunks of 128
NST = 16            # s-tiles of 128
QT = 512            # attention q tile
NQT = S // QT
KC = 128            # attention k chunk
RMS_EPS = 1.1920929e-07
SQRT_DH = float(np.sqrt(DH))
MAGIC = 0x5F3759DF


def _rsqrt_newton(nc, rs, ssq, tn, hn):
    """rs = rsqrt(ssq) elementwise via bit trick + 2 Newton iterations."""
    nc.vector.tensor_scalar(out=rs.bitcast(dt.int32), in0=ssq.bitcast(dt.int32),
                            scalar1=1, scalar2=None, op0=AO.logical_shift_right)
    nc.vector.tensor_scalar(out=rs.bitcast(dt.int32), in0=rs.bitcast(dt.int32),
                            scalar1=MAGIC, scalar2=-1, op0=AO.subtract, op1=AO.mult)
    nc.vector.tensor_scalar(out=hn, in0=ssq, scalar1=0.5, scalar2=None, op0=AO.mult)
    for _ in range(2):
        nc.vector.tensor_tensor(out=tn, in0=rs, in1=rs, op=AO.mult)
        nc.vector.tensor_tensor(out=tn, in0=tn, in1=hn, op=AO.mult)
        nc.vector.tensor_scalar(out=tn, in0=tn, scalar1=1.5, scalar2=-1.0,
                                op0=AO.subtract, op1=AO.mult)
        nc.vector.tensor_tensor(out=rs, in0=rs, in1=tn, op=AO.mult)


def build():
    nc = bacc.Bacc("TRN2", target_bir_lowering=False, debug=False, num_devices=N_CORES)

    xt = nc.dram_tensor("xt", [D, S], dt.bfloat16, kind="ExternalInput").ap()
    wqkv = nc.dram_tensor("wqkv", [D, 512], dt.bfloat16, kind="ExternalInput").ap()
    wot = nc.dram_tensor("wot", [D, HL * DH], dt.bfloat16, kind="ExternalInput").ap()
    # cbar/sbar: pair-interleave-expanded cos/sin, duplicated for both heads [S, 256]
    cbar = nc.dram_tensor("cbar", [S, HL * DH], dt.float32, kind="ExternalInput").ap()
    sbar = nc.dram_tensor("sbar", [S, HL * DH], dt.float32, kind="ExternalInput").ap()
    trimask = nc.dram_tensor("trimask", [KC, KC], dt.bfloat16, kind="ExternalInput").ap()
    esd = nc.dram_tensor("es", [128, HL], dt.float32, kind="ExternalInput").ap()
    y_out = nc.dram_tensor("y", [S, HL * DH], dt.float32, kind="ExternalOutput").ap()

    with tile.TileContext(nc) as tc:
        with (
            tc.tile_pool(name="const", bufs=1) as cp,
            tc.tile_pool(name="work", bufs=2) as wp,
            tc.tile_pool(name="psum", bufs=2, space="PSUM") as pp,
            tc.tile_pool(name="dram", bufs=1, space="DRAM") as dp,
        ):
            # ---- persistent tiles ----
            wqkv_sb = cp.tile([128, NC, 512], dt.bfloat16, tag="wqkv")
            xt_sb = cp.tile([128, NC, S], dt.bfloat16, tag="xt")
            wot_sb = cp.tile([128, NC, HL * DH], dt.bfloat16, tag="wot")
            cbar_sb = cp.tile([128, NST, HL * DH], dt.float32, tag="cbar")
            sbar_sb = cp.tile([128, NST, HL * DH], dt.float32, tag="sbar")
            tri_sb = cp.tile([128, KC], dt.bfloat16, tag="tri")
            es_sb = cp.tile([128, HL], dt.float32, tag="es")
            ones128 = cp.tile([128, 128], dt.bfloat16, tag="ones128")
            nc.vector.memset(ones128[:], 1.0)

            qT = cp.tile([128, HL, NST, 128], dt.bfloat16, tag="qT")  # [dh, h, st, s]
            kT = cp.tile([128, NST, 128], dt.bfloat16, tag="kT")      # [dh, st, s]
            v_sb = cp.tile([128, NST, DH], dt.bfloat16, tag="v")      # [s, st, dh]

            # ---- AllGather bounce buffers (one pair per q-group) ----
            ag_ins = [dp.tile([HL * 128, QT], dt.bfloat16, name=f"ag_in{i}")
                      for i in range(NQT)]
            ag_outs = [dp.tile([N_CORES * HL * 128, QT], dt.bfloat16, addr_space="Shared",
                               name=f"ag_out{i}") for i in range(NQT)]

            # ---- input DMA schedule (finest pieces first so PE starts early) ----
            wqr = wqkv.rearrange("(c p) e -> p c e", p=128)
            for g in range(4):
                eng = nc.sync if g % 2 == 0 else nc.scalar
                eng.dma_start(wqkv_sb[:, 4 * g:4 * g + 4, :], wqr[:, 4 * g:4 * g + 4, :])
            xtr = xt.rearrange("(c p) s -> p c s", p=128)
            nc.sync.dma_start(xt_sb[:, 0:8, 0:512], xtr[:, 0:8, 0:512])
            nc.scalar.dma_start(xt_sb[:, 8:16, 0:512], xtr[:, 8:16, 0:512])
            cbr = cbar.rearrange("(c p) e -> p c e", p=128)
            sbr = sbar.rearrange("(c p) e -> p c e", p=128)
            nc.sync.dma_start(cbar_sb[:, 0:4, :], cbr[:, 0:4, :])
            nc.scalar.dma_start(sbar_sb[:, 0:4, :], sbr[:, 0:4, :])
            nc.sync.dma_start(tri_sb[:], trimask)
            nc.sync.dma_start(es_sb[:], esd)
            for g in range(1, 4):
                gsl = slice(g * 512, (g + 1) * 512)
                nc.sync.dma_start(xt_sb[:, 0:8, gsl], xtr[:, 0:8, gsl])
                nc.scalar.dma_start(xt_sb[:, 8:16, gsl], xtr[:, 8:16, gsl])
            nc.sync.dma_start(cbar_sb[:, 4:10, :], cbr[:, 4:10, :])
            nc.scalar.dma_start(sbar_sb[:, 4:10, :], sbr[:, 4:10, :])
            nc.sync.dma_start(cbar_sb[:, 10:16, :], cbr[:, 10:16, :])
            nc.scalar.dma_start(sbar_sb[:, 10:16, :], sbr[:, 10:16, :])
            wotr = wot.rearrange("(c p) e -> p c e", p=128)
            nc.sync.dma_start(wot_sb[:, 0:8, :], wotr[:, 0:8, :])
            nc.scalar.dma_start(wot_sb[:, 8:16, :], wotr[:, 8:16, :])

            def proj(st):
                ssl = slice(st * 128, (st + 1) * 128)
                mm = pp.tile([128, 512], dt.float32, tag="mm")  # q[0:256] | k[256:384] | v[384:512]
                for c in range(NC):
                    nc.tensor.matmul(mm[:], xt_sb[:, c, ssl], wqkv_sb[:, c, :],
                                     start=(c == 0), stop=(c == NC - 1))

                # evacuate PSUM: q|k to f32 SBUF, v to bf16
                qk = wp.tile([128, 384], dt.float32, tag="qk", bufs=4)
                nc.vector.tensor_copy(qk[:], mm[:, 0:384])
                nc.vector.tensor_copy(v_sb[:, st, :], mm[:, 384:512])

                # sum of squares for q heads and k
                ssq = wp.tile([128, 4], dt.float32, tag="ssq")
                scr = wp.tile([128, 128], dt.float32, tag="scr")
                for i in range(3):
                    nc.scalar.activation(scr[:], qk[:, i * DH:(i + 1) * DH], AF.Square,
                                         accum_out=ssq[:, i:i + 1])

                # rs = rsqrt(ssq + 128*eps); cols 0,1 = q heads, col 2 = k
                rs = wp.tile([128, 4], dt.float32, tag="rs")
                tn = wp.tile([128, 4], dt.float32, tag="tn")
                hn = wp.tile([128, 4], dt.float32, tag="hn")
                nc.vector.tensor_scalar(out=ssq[:], in0=ssq[:], scalar1=128.0 * RMS_EPS,
                                        scalar2=None, op0=AO.add)
                _rsqrt_newton(nc, rs[:], ssq[:], tn[:], hn[:])
                # q scale: rsqrt(mean+eps) = rs * sqrt(128); k keeps rs (1/sqrt(dh) folded)
                nc.vector.tensor_scalar(out=rs[:, 0:2], in0=rs[:, 0:2], scalar1=SQRT_DH,
                                        scalar2=None, op0=AO.mult)

                # rope q (both heads in one set of ops; 3-D APs pair the heads)
                q3e = qk[:, 0:256].rearrange("p (h d) -> p h d", h=HL)[:, :, 0:DH:2]
                q3o = qk[:, 0:256].rearrange("p (h d) -> p h d", h=HL)[:, :, 1:DH:2]
                w = wp.tile([128, HL * DH], dt.float32, tag="w")
                w3 = w[:].rearrange("p (h d) -> p h d", h=HL)
                nc.vector.tensor_scalar(out=w3[:, :, 0:DH:2], in0=q3o, scalar1=-1.0,
                                        scalar2=None, op0=AO.mult)
                nc.vector.tensor_copy(w3[:, :, 1:DH:2], q3e)
                u1 = wp.tile([128, HL * DH], dt.float32, tag="u1")
                qhat = wp.tile([128, HL * DH], dt.bfloat16, tag="qhat")
                nc.vector.tensor_tensor(out=u1[:], in0=qk[:, 0:256], in1=cbar_sb[:, st, :],
                                        op=AO.mult)
                nc.vector.tensor_tensor(out=w[:], in0=w[:], in1=sbar_sb[:, st, :], op=AO.mult)
                nc.vector.tensor_add(out=qhat[:], in0=u1[:], in1=w[:])
                for h in range(HL):
                    nc.vector.tensor_scalar(out=qhat[:, h * DH:(h + 1) * DH],
                                            in0=qhat[:, h * DH:(h + 1) * DH],
                                            scalar1=rs[:, h:h + 1], scalar2=None, op0=AO.mult)

                # rope k
                kw = wp.tile([128, DH], dt.float32, tag="kw")
                ku = wp.tile([128, DH], dt.float32, tag="ku")
                khat = wp.tile([128, DH], dt.bfloat16, tag="khat")
                nc.vector.tensor_scalar(out=kw[:, 0:DH:2], in0=qk[:, 256 + 1:384:2],
                                        scalar1=-1.0, scalar2=None, op0=AO.mult)
                nc.vector.tensor_copy(kw[:, 1:DH:2], qk[:, 256 + 0:384:2])
                nc.vector.tensor_tensor(out=ku[:], in0=qk[:, 256:384],
                                        in1=cbar_sb[:, st, 0:DH], op=AO.mult)
                nc.vector.tensor_tensor(out=kw[:], in0=kw[:], in1=sbar_sb[:, st, 0:DH],
                                        op=AO.mult)
                nc.vector.tensor_add(out=ku[:], in0=ku[:], in1=kw[:])
                nc.vector.tensor_scalar(out=khat[:], in0=ku[:], scalar1=rs[:, 2:3],
                                        scalar2=None, op0=AO.mult)

                # XBAR DMA transposes -> qT / kT (off the PE)
                for h in range(HL):
                    nc.sync.dma_start(qT[:, h, st, :], qhat[:, h * DH:(h + 1) * DH],
                                      transpose=True)
                nc.sync.dma_start(kT[:, st, :], khat[:], transpose=True)

            def attn_group(t):
                # chunk i: i < 4t -> full k-chunk c=i over q cols [0:512)
                #          i >= 4t -> diagonal chunk c=4t+j over q cols [128j:512)
                nch = 4 * t + 4
                for h in range(HL):
                    lacc = pp.tile([128, QT], dt.float32, tag="lacc", bufs=1)
                    oacc = pp.tile([128, QT], dt.float32, tag="oacc", bufs=1)
                    pts = [None] * nch

                    def chunk_info(i):
                        if i < 4 * t:
                            return i, 0
                        j = i - 4 * t
                        return 4 * t + j, 128 * j

                    def emit_qk(i):
                        c, qoff = chunk_info(i)
                        w = QT - qoff
                        stp = pp.tile([128, QT], dt.float32, tag="stp", bufs=4)
                        nc.tensor.matmul(stp[:, 0:w], kT[:, c, :],
                                         qT[:, h, 4 * t + qoff // 128:4 * t + 4, :],
                                         start=True, stop=True)
                        pt = wp.tile([128, QT], dt.bfloat16, tag="pt", bufs=4)
                        nc.scalar.activation(pt[:, 0:w], stp[:, 0:w], AF.Exp)
                        if i >= 4 * t:
                            nc.vector.tensor_tensor(out=pt[:, 0:KC], in0=pt[:, 0:KC],
                                                    in1=tri_sb[:], op=AO.mult)
                        pts[i] = (pt, c, qoff, w)

                    def emit_acc(i, last):
                        pt, c, qoff, w = pts[i]
                        nc.tensor.matmul(lacc[:, qoff:QT], ones128[:], pt[:, 0:w],
                                         start=(i == 0), stop=last)
                        nc.tensor.matmul(oacc[:, qoff:QT], v_sb[:, c, :], pt[:, 0:w],
                                         start=(i == 0), stop=last)

                    emit_qk(0)
                    for i in range(1, nch):
                        emit_qk(i)
                        emit_acc(i - 1, last=False)
                    emit_acc(nch - 1, last=True)

                    # out = oacc / (lacc + exp(sink))
                    tmp = wp.tile([128, QT], dt.float32, tag="tmp")
                    nc.vector.tensor_scalar(out=tmp[:], in0=lacc[:],
                                            scalar1=es_sb[:, h:h + 1], scalar2=None,
                                            op0=AO.add)
                    rr = wp.tile([128, QT], dt.float32, tag="rr")
                    nc.vector.reciprocal_approx_fast(rr[:], tmp[:])
                    att = wp.tile([128, QT], dt.bfloat16, tag="att")
                    nc.vector.tensor_tensor(out=att[:], in0=oacc[:], in1=rr[:], op=AO.mult)
                    nc.scalar.dma_start(
                        ag_ins[t][:].rearrange("(h p) q -> p h q", p=128)[:, h, :], att[:])
                nc.gpsimd.collective_compute(
                    "AllGather", AO.bypass,
                    replica_groups=[list(range(N_CORES))],
                    ins=[ag_ins[t][:].opt()], outs=[ag_outs[t][:].opt()],
                )

            def wo_part(t):
                agr = ag_outs[t][:].rearrange("(c p) q -> p c q", p=128)
                for tt in range(QT // 128):
                    aT = wp.tile([128, NC, 128], dt.bfloat16, tag="aT", bufs=4)
                    nc.sync.dma_start(aT[:], agr[:, :, tt * 128:(tt + 1) * 128])
                    yp = pp.tile([128, 512], dt.float32, tag="mm")
                    for c in range(NC):
                        nc.tensor.matmul(yp[:, 0:HL * DH], aT[:, c, :], wot_sb[:, c, :],
                                         start=(c == 0), stop=(c == NC - 1))
                    ysb = wp.tile([128, HL * DH], dt.float32, tag="ysb")
                    nc.scalar.copy(ysb[:], yp[:, 0:HL * DH])
                    nc.sync.dma_start(y_out[t * QT + tt * 128:t * QT + (tt + 1) * 128, :],
                                      ysb[:])

            # ---- emission: all local work first, wo (collective-dependent) last ----
            for st in range(NST):
                proj(st)
                if st >= 4 and st % 4 == 0:
                    attn_group(st // 4 - 1)
            attn_group(NQT - 1)
            for t in range(NQT):
                wo_part(t)

    nc.compile()
    return nc


def prep_inputs(x, freqs_cis, wq, wk, wv, wo, sinks):
    """Host-side sharding/layout prep. Returns in_maps for the 8 cores."""
    x2 = np.ascontiguousarray(np.asarray(x, np.float32).reshape(S, D))
    xt = np.ascontiguousarray(x2.T).astype(BF16)
    fc = np.asarray(freqs_cis, np.float32)
    cos, sin = fc[:, :, 0], fc[:, :, 1]
    # pair-interleaved expansion duplicated for 2 heads: cbar[s, h*128 + 2j(+1)] = cos[s, j]
    cbar1 = np.repeat(cos, 2, axis=1)          # [S, 128]
    sbar1 = np.repeat(sin, 2, axis=1)
    cbar = np.tile(cbar1, (1, HL)).astype(np.float32)
    sbar = np.tile(sbar1, (1, HL)).astype(np.float32)

    kr = np.arange(KC)[:, None]
    qr = np.arange(KC)[None, :]
    trimask = (qr >= kr).astype(np.float32).astype(BF16)   # [128, 128]

    wq = np.asarray(wq, np.float32)
    wk = np.asarray(wk, np.float32)
    wv = np.asarray(wv, np.float32)
    wo = np.asarray(wo, np.float32)
    sinks = np.asarray(sinks, np.float32)

    in_maps = []
    for d in range(N_CORES):
        kv = d // 2
        es = np.exp(sinks[2 * d:2 * d + 2]).astype(np.float32)
        wqkv = np.concatenate([
            wq[d * 256:(d + 1) * 256, :].T,
            wk[kv * 128:(kv + 1) * 128, :].T,
            wv[kv * 128:(kv + 1) * 128, :].T,
        ], axis=1)
        in_maps.append({
            "xt": xt,
            "wqkv": np.ascontiguousarray(wqkv).astype(BF16),
            "wot": np.ascontiguousarray(wo[d * 256:(d + 1) * 256, :].T).astype(BF16),
            "cbar": cbar,
            "sbar": sbar,
            "trimask": trimask,
            "es": np.repeat(es[None, :], 128, axis=0).astype(np.float32),
        })
    return in_maps


_CACHED = {}


def kernel(x, freqs_cis, wq, wk, wv, wo, sinks):
    if "nc" not in _CACHED:
        _CACHED["nc"] = build()
    nc = _CACHED["nc"]
    in_maps = prep_inputs(x, freqs_cis, wq, wk, wv, wo, sinks)
    res = run_bass_kernel_spmd(nc, in_maps, list(range(N_CORES)), trace=False)
    y = np.concatenate([res.results[d]["y"] for d in range(N_CORES)], axis=1)
    return y.reshape(1, S, D).astype(np.float32)


# revision 3
# speedup vs baseline: 1.3646x; 1.3646x over previous
"""Distributed Bass kernel for nn_Attention_32701880992127 on 8 TRN2 NeuronCores.

Sharding (tensor parallel over heads): core d owns q-heads {2d, 2d+1} and
kv-head d//2 (GQA consecutive-repeat mapping). wq/wk/wv are column-split,
wo is split along its OUTPUT dim so each core produces a distinct column
slice of the final output from the AllGathered attention features.

All matmuls run in bf16 (f32 PSUM accumulation); elementwise math stays f32.
Softmax needs no max-subtraction (qk-norm bounds the scores); the sink
correction folds into the denominator:
    out_h = (sum_k exp(s_qk) v_k) / (exp(sink_h) + sum_k exp(s_qk)).
Scores are computed transposed (ST[k, q]) so exp's output directly feeds the
PV matmul as the moving operand. The causal diagonal 512-block is processed
as 4 k-chunks with shrinking q-windows; only the 128x128 diagonal block of
each chunk needs a (shared triangular) mask.

All DRAM inputs are pre-tiled on the host into partition-major layouts so
every DMA moves >=4KB contiguous runs per partition (the DMA queues are
packet-rate limited: 256B-1KB runs cap them at ~15-50GB/s). Emission order
keeps the PE FIFO free of collective-dependent work until all local
attention is done; AllGathers fire as soon as each q-group ships.
"""
import numpy as np
import ml_dtypes

import concourse.mybir as mybir
import concourse.tile as tile
from concourse import bacc
from concourse.bass_utils import run_bass_kernel_spmd
from concourse.masks import make_identity

dt = mybir.dt
AO = mybir.AluOpType
AF = mybir.ActivationFunctionType
BF16 = ml_dtypes.bfloat16

N_CORES = 8
S = 2048            # sequence length
D = 2048            # model dim
DH = 128            # head dim
HL = 2              # local q heads per core
NC = 16             # d-chunks of 128
NST = 16            # s-tiles of 128
QT = 512            # attention q tile
NQT = S // QT
KC = 128            # attention k chunk
RMS_EPS = 1.1920929e-07
SQRT_DH = float(np.sqrt(DH))
MAGIC = 0x5F3759DF


def _rsqrt_newton(nc, rs, ssq, tn, hn):
    """rs = rsqrt(ssq) elementwise via bit trick + 2 Newton iterations."""
    nc.vector.tensor_scalar(out=rs.bitcast(dt.int32), in0=ssq.bitcast(dt.int32),
                            scalar1=1, scalar2=None, op0=AO.logical_shift_right)
    nc.vector.tensor_scalar(out=rs.bitcast(dt.int32), in0=rs.bitcast(dt.int32),
                            scalar1=MAGIC, scalar2=-1, op0=AO.subtract, op1=AO.mult)
    nc.vector.tensor_scalar(out=hn, in0=ssq, scalar1=0.5, scalar2=None, op0=AO.mult)
    for _ in range(2):
        nc.vector.tensor_tensor(out=tn, in0=rs, in1=rs, op=AO.mult)
        nc.vector.tensor_tensor(out=tn, in0=tn, in1=hn, op=AO.mult)
        nc.vector.tensor_scalar(out=tn, in0=tn, scalar1=1.5, scalar2=-1.0,
                                op0=AO.subtract, op1=AO.mult)
        nc.vector.tensor_tensor(out=rs, in0=rs, in1=tn, op=AO.mult)


def build():
    nc = bacc.Bacc("TRN2", target_bir_lowering=False, debug=False, num_devices=N_CORES)

    # all inputs pre-tiled partition-major on the host: [p, ...] with long
    # contiguous per-partition runs
    xt = nc.dram_tensor("xt", [128, NST * NC * 128], dt.bfloat16,
                        kind="ExternalInput").ap()            # [p, st, c, s]
    wqkv = nc.dram_tensor("wqkv", [128, NC * 512], dt.bfloat16,
                          kind="ExternalInput").ap()          # [p, c, e]
    wot = nc.dram_tensor("wot", [128, NC * HL * DH], dt.bfloat16,
                         kind="ExternalInput").ap()           # [p, c, e]
    cbar = nc.dram_tensor("cbar", [128, NST * HL * DH], dt.float32,
                          kind="ExternalInput").ap()          # [p, st, e]
    sbar = nc.dram_tensor("sbar", [128, NST * HL * DH], dt.float32,
                          kind="ExternalInput").ap()
    trimask = nc.dram_tensor("trimask", [KC, KC], dt.bfloat16, kind="ExternalInput").ap()
    esd = nc.dram_tensor("es", [128, HL], dt.float32, kind="ExternalInput").ap()
    y_out = nc.dram_tensor("y", [S, HL * DH], dt.float32, kind="ExternalOutput").ap()

    with tile.TileContext(nc) as tc:
        with (
            tc.tile_pool(name="const", bufs=1) as cp,
            tc.tile_pool(name="work", bufs=2) as wp,
            tc.tile_pool(name="psum", bufs=2, space="PSUM") as pp,
            tc.tile_pool(name="dram", bufs=1, space="DRAM") as dp,
        ):
            # ---- persistent tiles ----
            wqkv_sb = cp.tile([128, NC, 512], dt.bfloat16, tag="wqkv")
            xt_sb = cp.tile([128, NST, NC, 128], dt.bfloat16, tag="xt")
            wot_sb = cp.tile([128, NC, HL * DH], dt.bfloat16, tag="wot")
            cbar_sb = cp.tile([128, NST, HL * DH], dt.float32, tag="cbar")
            sbar_sb = cp.tile([128, NST, HL * DH], dt.float32, tag="sbar")
            tri_sb = cp.tile([128, KC], dt.bfloat16, tag="tri")
            es_sb = cp.tile([128, HL], dt.float32, tag="es")
            ones128 = cp.tile([128, 128], dt.bfloat16, tag="ones128")
            nc.vector.memset(ones128[:], 1.0)
            ident = cp.tile([128, 128], dt.bfloat16, tag="ident")
            make_identity(nc, ident[:])

            qT = cp.tile([128, HL, NST, 128], dt.bfloat16, tag="qT")  # [dh, h, st, s]
            kT = cp.tile([128, NST, 128], dt.bfloat16, tag="kT")      # [dh, st, s]
            v_sb = cp.tile([128, NST, DH], dt.bfloat16, tag="v")      # [s, st, dh]

            # ---- AllGather bounce buffers (one pair per q-group) ----
            ag_ins = [dp.tile([HL * 128, QT], dt.bfloat16, name=f"ag_in{i}")
                      for i in range(NQT)]
            ag_outs = [dp.tile([N_CORES * HL * 128, QT], dt.bfloat16, addr_space="Shared",
                               name=f"ag_out{i}") for i in range(NQT)]

            # ---- input DMA schedule: all pieces are contiguous per partition ----
            # scalar queue: xt st0, st1, rope tables, xt odd tiles, es, wot
            # sync queue:   wqkv, xt even tiles, trimask
            xts = xt.rearrange("p (st e) -> p st e", st=NST)
            nc.scalar.dma_start(xt_sb[:, 0, :, :], xts[:, 0, :])
            nc.scalar.dma_start(xt_sb[:, 1, :, :], xts[:, 1, :])
            wqr = wqkv.rearrange("p (c e) -> p c e", c=NC)
            for g in range(4):
                nc.sync.dma_start(wqkv_sb[:, 4 * g:4 * g + 4, :], wqr[:, 4 * g:4 * g + 4, :])
            nc.scalar.dma_start(cbar_sb[:], cbar.rearrange("p (st e) -> p st e", st=NST))
            nc.scalar.dma_start(sbar_sb[:], sbar.rearrange("p (st e) -> p st e", st=NST))
            for st in range(2, NST):
                eng = nc.sync if st % 2 == 0 else nc.scalar
                eng.dma_start(xt_sb[:, st, :, :], xts[:, st, :])
            nc.sync.dma_start(tri_sb[:], trimask)
            nc.scalar.dma_start(es_sb[:], esd)
            nc.scalar.dma_start(wot_sb[:], wot.rearrange("p (c e) -> p c e", c=NC))

            def proj(st):
                mm = pp.tile([128, 512], dt.float32, tag="mm")  # q[0:256] | k[256:384] | v[384:512]
                for c in range(NC):
                    nc.tensor.matmul(mm[:], xt_sb[:, st, c, :], wqkv_sb[:, c, :],
                                     start=(c == 0), stop=(c == NC - 1))

                # evacuate PSUM: q|k to f32 SBUF, v to bf16
                qk = wp.tile([128, 384], dt.float32, tag="qk", bufs=4)
                nc.vector.tensor_copy(qk[:], mm[:, 0:384])
                nc.vector.tensor_copy(v_sb[:, st, :], mm[:, 384:512])

                # sum of squares for q heads and k
                ssq = wp.tile([128, 4], dt.float32, tag="ssq")
                scr = wp.tile([128, 128], dt.float32, tag="scr")
                for i in range(3):
                    nc.scalar.activation(scr[:], qk[:, i * DH:(i + 1) * DH], AF.Square,
                                         accum_out=ssq[:, i:i + 1])

                # rs = rsqrt(ssq + 128*eps); cols 0,1 = q heads, col 2 = k
                rs = wp.tile([128, 4], dt.float32, tag="rs")
                tn = wp.tile([128, 4], dt.float32, tag="tn")
                hn = wp.tile([128, 4], dt.float32, tag="hn")
                nc.vector.tensor_scalar(out=ssq[:], in0=ssq[:], scalar1=128.0 * RMS_EPS,
                                        scalar2=None, op0=AO.add)
                _rsqrt_newton(nc, rs[:], ssq[:], tn[:], hn[:])
                # q scale: rsqrt(mean+eps) = rs * sqrt(128); k keeps rs (1/sqrt(dh) folded)
                nc.vector.tensor_scalar(out=rs[:, 0:2], in0=rs[:, 0:2], scalar1=SQRT_DH,
                                        scalar2=None, op0=AO.mult)

                # rope q (both heads in one set of ops; 3-D APs pair the heads)
                q3e = qk[:, 0:256].rearrange("p (h d) -> p h d", h=HL)[:, :, 0:DH:2]
                q3o = qk[:, 0:256].rearrange("p (h d) -> p h d", h=HL)[:, :, 1:DH:2]
                w = wp.tile([128, HL * DH], dt.float32, tag="w")
                w3 = w[:].rearrange("p (h d) -> p h d", h=HL)
                nc.vector.tensor_scalar(out=w3[:, :, 0:DH:2], in0=q3o, scalar1=-1.0,
                                        scalar2=None, op0=AO.mult)
                nc.vector.tensor_copy(w3[:, :, 1:DH:2], q3e)
                u1 = wp.tile([128, HL * DH], dt.float32, tag="u1")
                qhat = wp.tile([128, HL * DH], dt.bfloat16, tag="qhat")
                nc.vector.tensor_tensor(out=u1[:], in0=qk[:, 0:256], in1=cbar_sb[:, st, :],
                                        op=AO.mult)
                nc.vector.tensor_tensor(out=w[:], in0=w[:], in1=sbar_sb[:, st, :], op=AO.mult)
                nc.vector.tensor_add(out=qhat[:], in0=u1[:], in1=w[:])
                for h in range(HL):
                    nc.vector.tensor_scalar(out=qhat[:, h * DH:(h + 1) * DH],
                                            in0=qhat[:, h * DH:(h + 1) * DH],
                                            scalar1=rs[:, h:h + 1], scalar2=None, op0=AO.mult)

                # rope k
                kw = wp.tile([128, DH], dt.float32, tag="kw")
                ku = wp.tile([128, DH], dt.float32, tag="ku")
                khat = wp.tile([128, DH], dt.bfloat16, tag="khat")
                nc.vector.tensor_scalar(out=kw[:, 0:DH:2], in0=qk[:, 256 + 1:384:2],
                                        scalar1=-1.0, scalar2=None, op0=AO.mult)
                nc.vector.tensor_copy(kw[:, 1:DH:2], qk[:, 256 + 0:384:2])
                nc.vector.tensor_tensor(out=ku[:], in0=qk[:, 256:384],
                                        in1=cbar_sb[:, st, 0:DH], op=AO.mult)
                nc.vector.tensor_tensor(out=kw[:], in0=kw[:], in1=sbar_sb[:, st, 0:DH],
                                        op=AO.mult)
                nc.vector.tensor_add(out=ku[:], in0=ku[:], in1=kw[:])
                nc.vector.tensor_scalar(out=khat[:], in0=ku[:], scalar1=rs[:, 2:3],
                                        scalar2=None, op0=AO.mult)

                # PE transposes -> qT / kT (PSUM copies on ACT)
                for h in range(HL):
                    tp = pp.tile([128, 128], dt.bfloat16, tag="tp")
                    nc.tensor.transpose(tp[:], qhat[:, h * DH:(h + 1) * DH], ident[:])
                    nc.scalar.copy(qT[:, h, st, :], tp[:])
                tpk = pp.tile([128, 128], dt.bfloat16, tag="tp")
                nc.tensor.transpose(tpk[:], khat[:], ident[:])
                nc.scalar.copy(kT[:, st, :], tpk[:])

            def attn_group(t):
                # chunk i: i < 4t -> full k-chunk c=i over q cols [0:512)
                #          i >= 4t -> diagonal chunk c=4t+j over q cols [128j:512)
                nch = 4 * t + 4
                for h in range(HL):
                    lacc = pp.tile([128, QT], dt.float32, tag="lacc", bufs=1)
                    oacc = pp.tile([128, QT], dt.float32, tag="oacc", bufs=1)
                    pts = [None] * nch

                    def chunk_info(i):
                        if i < 4 * t:
                            return i, 0
                        j = i - 4 * t
                        return 4 * t + j, 128 * j

                    def emit_qk(i):
                        c, qoff = chunk_info(i)
                        w = QT - qoff
                        stp = pp.tile([128, QT], dt.float32, tag="stp")
                        nc.tensor.matmul(stp[:, 0:w], kT[:, c, :],
                                         qT[:, h, 4 * t + qoff // 128:4 * t + 4, :],
                                         start=True, stop=True)
                        pt = wp.tile([128, QT], dt.bfloat16, tag="pt", bufs=4)
                        nc.scalar.activation(pt[:, 0:w], stp[:, 0:w], AF.Exp)
                        if i >= 4 * t:
                            nc.vector.tensor_tensor(out=pt[:, 0:KC], in0=pt[:, 0:KC],
                                                    in1=tri_sb[:], op=AO.mult)
                        pts[i] = (pt, c, qoff, w)

                    def emit_acc(i, last):
                        pt, c, qoff, w = pts[i]
                        nc.tensor.matmul(lacc[:, qoff:QT], ones128[:], pt[:, 0:w],
                                         start=(i == 0), stop=last)
                        nc.tensor.matmul(oacc[:, qoff:QT], v_sb[:, c, :], pt[:, 0:w],
                                         start=(i == 0), stop=last)

                    emit_qk(0)
                    for i in range(1, nch):
                        emit_qk(i)
                        emit_acc(i - 1, last=False)
                    emit_acc(nch - 1, last=True)

                    # out = oacc / (lacc + exp(sink))
                    tmp = wp.tile([128, QT], dt.float32, tag="tmp")
                    nc.vector.tensor_scalar(out=tmp[:], in0=lacc[:],
                                            scalar1=es_sb[:, h:h + 1], scalar2=None,
                                            op0=AO.add)
                    rr = wp.tile([128, QT], dt.float32, tag="rr")
                    nc.vector.reciprocal_approx_fast(rr[:], tmp[:])
                    att = wp.tile([128, QT], dt.bfloat16, tag="att")
                    nc.vector.tensor_tensor(out=att[:], in0=oacc[:], in1=rr[:], op=AO.mult)
                    nc.scalar.dma_start(
                        ag_ins[t][:].rearrange("(h p) q -> p h q", p=128)[:, h, :], att[:])
                nc.gpsimd.collective_compute(
                    "AllGather", AO.bypass,
                    replica_groups=[list(range(N_CORES))],
                    ins=[ag_ins[t][:].opt()], outs=[ag_outs[t][:].opt()],
                )

            def wo_part(t):
                agr = ag_outs[t][:].rearrange("(c p) q -> p c q", p=128)
                aT = wp.tile([128, NC, QT], dt.bfloat16, tag="aT", bufs=2)
                nc.sync.dma_start(aT[:, 0:8, :], agr[:, 0:8, :])
                nc.scalar.dma_start(aT[:, 8:16, :], agr[:, 8:16, :])
                for tt in range(QT // 128):
                    qsl = slice(tt * 128, (tt + 1) * 128)
                    yp = pp.tile([128, 512], dt.float32, tag="mm")
                    for c in range(NC):
                        nc.tensor.matmul(yp[:, 0:HL * DH], aT[:, c, qsl], wot_sb[:, c, :],
                                         start=(c == 0), stop=(c == NC - 1))
                    ysb = wp.tile([128, HL * DH], dt.float32, tag="ysb")
                    nc.scalar.copy(ysb[:], yp[:, 0:HL * DH])
                    nc.sync.dma_start(y_out[t * QT + tt * 128:t * QT + (tt + 1) * 128, :],
                                      ysb[:])

            # ---- emission: all local work first, wo (collective-dependent) last ----
            for st in range(NST):
                proj(st)
                if st >= 4 and st % 4 == 0:
                    attn_group(st // 4 - 1)
            attn_group(NQT - 1)
            for t in range(NQT):
                wo_part(t)

    nc.compile()
    return nc


def prep_inputs(x, freqs_cis, wq, wk, wv, wo, sinks):
    """Host-side sharding/layout prep. Returns in_maps for the 8 cores.

    All tensors are pre-tiled partition-major ([p, ...]) so DMAs move
    long contiguous per-partition runs.
    """
    x2 = np.ascontiguousarray(np.asarray(x, np.float32).reshape(S, D))
    xt = x2.T.astype(BF16)                                    # [D, S] = [(c p), (st s)]
    xt_h = np.ascontiguousarray(
        xt.reshape(NC, 128, NST, 128).transpose(1, 2, 0, 3).reshape(128, NST * NC * 128))

    fc = np.asarray(freqs_cis, np.float32)
    cos, sin = fc[:, :, 0], fc[:, :, 1]
    cbar1 = np.repeat(cos, 2, axis=1)          # [S, 128] pair-interleaved
    sbar1 = np.repeat(sin, 2, axis=1)
    cbar = np.tile(cbar1, (1, HL)).astype(np.float32)         # [S, 256]
    sbar = np.tile(sbar1, (1, HL)).astype(np.float32)
    cbar_h = np.ascontiguousarray(
        cbar.reshape(NST, 128, HL * DH).transpose(1, 0, 2).reshape(128, NST * HL * DH))
    sbar_h = np.ascontiguousarray(
        sbar.reshape(NST, 128, HL * DH).transpose(1, 0, 2).reshape(128, NST * HL * DH))

    kr = np.arange(KC)[:, None]
    qr = np.arange(KC)[None, :]
    trimask = (qr >= kr).astype(np.float32).astype(BF16)      # [128, 128]

    wq = np.asarray(wq, np.float32)
    wk = np.asarray(wk, np.float32)
    wv = np.asarray(wv, np.float32)
    wo = np.asarray(wo, np.float32)
    sinks = np.asarray(sinks, np.float32)

    in_maps = []
    for d in range(N_CORES):
        kv = d // 2
        es = np.exp(sinks[2 * d:2 * d + 2]).astype(np.float32)
        wqkv = np.concatenate([
            wq[d * 256:(d + 1) * 256, :].T,
            wk[kv * 128:(kv + 1) * 128, :].T,
            wv[kv * 128:(kv + 1) * 128, :].T,
        ], axis=1).astype(BF16)                               # [D, 512] = [(c p), e]
        wqkv_h = np.ascontiguousarray(
            wqkv.reshape(NC, 128, 512).transpose(1, 0, 2).reshape(128, NC * 512))
        wotd = np.ascontiguousarray(wo[d * 256:(d + 1) * 256, :].T).astype(BF16)
        wot_h = np.ascontiguousarray(
            wotd.reshape(NC, 128, HL * DH).transpose(1, 0, 2).reshape(128, NC * HL * DH))
        in_maps.append({
            "xt": xt_h,
            "wqkv": wqkv_h,
            "wot": wot_h,
            "cbar": cbar_h,
            "sbar": sbar_h,
            "trimask": trimask,
            "es": np.repeat(es[None, :], 128, axis=0).astype(np.float32),
        })
    return in_maps


_CACHED = {}


def kernel(x, freqs_cis, wq, wk, wv, wo, sinks):
    if "nc" not in _CACHED:
        _CACHED["nc"] = build()
    nc = _CACHED["nc"]
    in_maps = prep_inputs(x, freqs_cis, wq, wk, wv, wo, sinks)
    res = run_bass_kernel_spmd(nc, in_maps, list(range(N_CORES)), trace=False)
    y = np.concatenate([res.results[d]["y"] for d in range(N_CORES)], axis=1)
    return y.reshape(1, S, D).astype(np.float32)
